# revision 5
# baseline (speedup 1.0000x reference)
"""GateRow kernel for Trainium2 (8 NeuronCores, SPMD gate-sharded).

Problem: out[b, g] = gates[g, 2*x[b, c0[g]] + x[b, c1[g]]]
  x: [16384, 8192] bool, gates: [8192, 4] bool, choices: [8192, 2] int32.

Strategy (per core, gate-sharded GPC=1024, batch bit-packed):
  host:  pack the batch axis 8 bits/byte: TAB = [packbits(x)^T ; ~packbits(x)^T ;
         ones ; zeros]  (16386 rows x 2048 bytes).  Classify each gate's
         truth table into the universal form  f = (a' & b') ^ c'  where
         a'/b'/c' are TAB rows (possibly complemented wires or constants);
         this covers all 16 two-input boolean functions.
  device (per 256-gate chunk):
    1. dma_gather 768 rows (256 a' + 256 b' + 256 c') -> [128, 6, 2048] u8
    2. DVE bitwise AND:  q = a' & b'      (as uint32 lanes)
    3. DVE bitwise XOR:  f = q ^ c'       (as uint32 lanes)
    4. DMA out packed rows [128, 2, 2048] -> HBM
  host:  unpack bits + transpose to [B, G] bool.

Per-core HBM/DMA traffic: 6 MB gather + 2 MB out (vs 48 MB for the
byte-wise batch-sharded design) -- ~20-30 us/core expected.
"""

import sys

for _p in ("/opt/trn_rl_repo", "/opt/pypackages"):
    if _p not in sys.path:
        sys.path.append(_p)

from contextlib import ExitStack
from itertools import product

import numpy as np

import concourse.bass as bass  # noqa: F401  (registers engines)
import concourse.bacc as bacc
import concourse.tile as tile
import concourse.mybir as mybir
from concourse.bass_utils import run_bass_kernel_spmd

B, N, G, NCORES = 16384, 8192, 8192, 8
GPC = G // NCORES      # 1024 gates per core
BPACK = B // 8         # 2048 packed bytes per table row
NCHUNK = 4             # pipeline chunks per core
CG = GPC // NCHUNK     # 256 gates per chunk
NIDX = 3 * CG          # 768 gathered rows per chunk (a', b', c')
ROWS = 2 * N + 2       # x rows, ~x rows, ones, zeros

# ---------------------------------------------------------------------------
# Gate classification:  f(a,b) = (a' & b') ^ c'
#   selector codes: 0 = a, 1 = ~a, 2 = b, 3 = ~b, 4 = ones, 5 = zeros
# ---------------------------------------------------------------------------


def _classify_gates():
    def val(sel, a, b):
        return [a, 1 - a, b, 1 - b, 1, 0][sel]

    forms = np.zeros((16, 3), dtype=np.int64)
    for tt in range(16):
        found = False
        for sa, sb, sc in product([0, 1, 4, 5], [2, 3, 4, 5], range(6)):
            if all(
                ((val(sa, a, b) & val(sb, a, b)) ^ val(sc, a, b))
                == ((tt >> (2 * a + b)) & 1)
                for a in (0, 1)
                for b in (0, 1)
            ):
                forms[tt] = (sa, sb, sc)
                found = True
                break
        assert found, f"truth table {tt} not representable"
    return forms


_FORMS = _classify_gates()

# ---------------------------------------------------------------------------
# Device program (SPMD; all cores run it on their own 1024-gate shard)
# ---------------------------------------------------------------------------


def build_nc():
    u32 = mybir.dt.uint32
    pcall = NIDX // 16  # int16 idx slots per partition per gather call

    nc = bacc.Bacc(
        "TRN2", target_bir_lowering=False, debug=False, num_devices=NCORES
    )
    tab = nc.dram_tensor("tab", [ROWS, BPACK], mybir.dt.uint8, kind="ExternalInput")
    idxs = nc.dram_tensor(
        "idxs", [128, NCHUNK * pcall], mybir.dt.int16, kind="ExternalInput"
    )
    outd = nc.dram_tensor(
        "out", [128, (GPC // 128) * BPACK], mybir.dt.uint8, kind="ExternalOutput"
    )

    with tile.TileContext(nc) as tc, ExitStack() as ctx:
        pconst = ctx.enter_context(tc.tile_pool(name="const", bufs=1))
        pg = ctx.enter_context(tc.tile_pool(name="gather", bufs=4))
        pq = ctx.enter_context(tc.tile_pool(name="and", bufs=2))
        po = ctx.enter_context(tc.tile_pool(name="out", bufs=2))

        # Warm-up gather (16 dummy rows): pays the one-time Q7 ucode load
        # for the SWDGE gather path concurrently with the idx DMA below.
        widx_t = pconst.tile([128, 1], mybir.dt.int16)
        nc.gpsimd.memset(widx_t[:], 0)
        wout_t = pconst.tile([128, 1, BPACK], mybir.dt.uint8)
        nc.gpsimd.dma_gather(
            wout_t[:], tab[:], widx_t[:], 16, 16, BPACK, single_packet=True
        )

        idx_t = pconst.tile([128, idxs.shape[1]], mybir.dt.int16)
        nc.sync.dma_start(idx_t[:], idxs[:])

        for k in range(NCHUNK):
            g_t = pg.tile([128, 6, BPACK], mybir.dt.uint8, tag="g")
            nc.gpsimd.dma_gather(
                g_t[:],
                tab[:],
                idx_t[:, k * pcall : (k + 1) * pcall],
                NIDX,
                NIDX,
                BPACK,
                single_packet=True,
            )
            q_t = pq.tile([128, 2, BPACK], mybir.dt.uint8, tag="q")
            nc.vector.tensor_tensor(
                q_t[:].bitcast(u32),
                g_t[:, 0:2, :].bitcast(u32),
                g_t[:, 2:4, :].bitcast(u32),
                mybir.AluOpType.bitwise_and,
            )
            o_t = po.tile([128, 2, BPACK], mybir.dt.uint8, tag="o")
            nc.vector.tensor_tensor(
                o_t[:].bitcast(u32),
                q_t[:].bitcast(u32),
                g_t[:, 4:6, :].bitcast(u32),
                mybir.AluOpType.bitwise_xor,
            )
            nc.sync.dma_start(
                outd[:, k * 2 * BPACK : (k + 1) * 2 * BPACK], o_t[:]
            )
    nc.compile()
    return nc


# ---------------------------------------------------------------------------
# Host-side input prep / output assembly
# ---------------------------------------------------------------------------


def _prep_inputs(x, gates, choices):
    x8 = np.asarray(x, dtype=np.uint8)
    g8 = np.asarray(gates, dtype=np.uint8)
    ch = np.asarray(choices, dtype=np.int64)

    # table: packed x^T, complemented rows, ones, zeros
    xp = np.packbits(x8, axis=0, bitorder="little")  # [B/8, N] -> bit j of [k, w] = x[8k+j, w]
    tabx = np.ascontiguousarray(xp.T)                # [N, BPACK]
    tab_full = np.empty((ROWS, BPACK), dtype=np.uint8)
    tab_full[:N] = tabx
    tab_full[N : 2 * N] = tabx ^ 0xFF
    tab_full[2 * N] = 0xFF
    tab_full[2 * N + 1] = 0

    # per-gate row selectors
    tt = (g8[:, 0] | (g8[:, 1] << 1) | (g8[:, 2] << 2) | (g8[:, 3] << 3)).astype(
        np.int64
    )
    sel = _FORMS[tt]  # [G, 3]
    c0, c1 = ch[:, 0], ch[:, 1]

    def row_of(code):
        return np.select(
            [code == 0, code == 1, code == 2, code == 3, code == 4, code == 5],
            [c0, N + c0, c1, N + c1,
             np.full(G, 2 * N, np.int64), np.full(G, 2 * N + 1, np.int64)],
        )

    rowA, rowB, rowC = (row_of(sel[:, j]) for j in range(3))

    # dma_gather wrapped index layout: idx i -> partition i%16, slot i//16,
    # replicated across the 8 gpsimd cores (x8 partitions).
    in_maps = []
    for c in range(NCORES):
        cols = []
        for k in range(NCHUNK):
            gl = slice(c * GPC + k * CG, c * GPC + (k + 1) * CG)
            flat = np.concatenate([rowA[gl], rowB[gl], rowC[gl]]).astype(np.int16)
            wrapped = flat.reshape(-1, 16).T  # [16, NIDX/16]
            cols.append(np.tile(wrapped, (8, 1)))  # [128, NIDX/16]
        idxs_np = np.ascontiguousarray(np.concatenate(cols, axis=1))
        in_maps.append({"tab": tab_full, "idxs": idxs_np})
    return in_maps


def _assemble(results):
    parts = []
    for c in range(NCORES):
        o = results[c]["out"]  # [128, 8*BPACK] u8; row p, chunk k, slot j
        parts.append(
            o.reshape(128, NCHUNK, 2, BPACK)
            .transpose(1, 2, 0, 3)
            .reshape(GPC, BPACK)
        )
    packed = np.concatenate(parts, axis=0)  # [G, BPACK], gate-major
    return np.unpackbits(packed, axis=1, bitorder="little").T.astype(bool)


# ---------------------------------------------------------------------------
# Entry point
# ---------------------------------------------------------------------------

_NC_CACHE = {}


def _get_nc():
    if "nc" not in _NC_CACHE:
        _NC_CACHE["nc"] = build_nc()
    return _NC_CACHE["nc"]


def kernel(x, gates, choices):
    in_maps = _prep_inputs(x, gates, choices)
    nc = _get_nc()
    res = run_bass_kernel_spmd(nc, in_maps, list(range(NCORES)))
    return _assemble(res.results)


# revision 6
# speedup vs baseline: 1.1909x; 1.1909x over previous
"""GateRow kernel for Trainium2 (8 NeuronCores, SPMD gate-sharded).

Problem: out[b, g] = gates[g, 2*x[b, c0[g]] + x[b, c1[g]]]
  x: [16384, 8192] bool, gates: [8192, 4] bool, choices: [8192, 2] int32.

Strategy (per core, gate-sharded GPC=1024, batch bit-packed 8 bits/byte):
  host:  TAB = [packbits(x)^T ; ~packbits(x)^T ; ones ; zeros]
         (16386 rows x 2048 bytes).  Classify each gate:
           AND/OR class (14/16 truth tables): f = (a' & b') ^ m,
             a'/b' TAB rows, m per-gate constant byte mask (0x00/0xFF)
           XOR class (tt 0110/1001): f = a' ^ b'
         XOR-class gates are moved to dedicated tail slots per core.
  device:
    normal slots:  dma_gather a'|b' rows -> q = a'&b' -> o = q ^ mask
                   (mask via stride-0 broadcast of a per-partition u32)
    tail slots:    dma_gather a'|b' rows -> o = a'^b'
    all bitwise ops as uint32 lanes on DVE; packed rows DMA'd out.
  host:  unpack bits + transpose to [B, G] bool.

Descriptor count ~2.3k/core (was 3.1k in the 3-row universal form); DMA
~6.7 MB/core.  Pool-engine SWDGE descriptor generation is the serial
backbone, so index arrays are equalized across cores (shared
num_idxs_reg) with trailing -1 entries that generate no descriptors.
"""

import sys

for _p in ("/opt/trn_rl_repo", "/opt/pypackages"):
    if _p not in sys.path:
        sys.path.append(_p)

from contextlib import ExitStack
from itertools import product

import numpy as np

import concourse.bass as bass  # noqa: F401
import concourse.bacc as bacc
import concourse.tile as tile
import concourse.mybir as mybir
from concourse.bass_utils import run_bass_kernel_spmd

B, N, G, NCORES = 16384, 8192, 8192, 8
GPC = G // NCORES      # 1024 gates per core
BPACK = B // 8         # 2048 packed bytes per table row
ROWS = 2 * N + 2       # x rows, ~x rows, ones, zeros
ZROW = 2 * N + 1       # all-zeros row (pad target)

# normal-region calls: (slot_start, n_slots); tail call appended at build
NORM_CALLS = ((0, 2), (2, 3), (5, 3))

# ---------------------------------------------------------------------------
# Gate classification
#   AND/OR class: f(a,b) = (a' & b') ^ m,  a' in {a,~a,1,0}, b' in {b,~b,1,0}
#   XOR class (tt 6/9): f = a' ^ b'
#   selector codes: 0 = a, 1 = ~a, 2 = b, 3 = ~b, 4 = ones, 5 = zeros
# ---------------------------------------------------------------------------


def _classify_gates():
    def val(sel, a, b):
        return [a, 1 - a, b, 1 - b, 1, 0][sel]

    forms = np.full((16, 3), -1, dtype=np.int64)  # (sa, sb, m) ; m==2 -> XOR class
    for tt in range(16):
        if tt in (6, 9):
            # a ^ b   /   ~a ^ b
            forms[tt] = (0 if tt == 6 else 1, 2, 2)
            continue
        found = False
        for sa, sb, m in product([0, 1, 4, 5], [2, 3, 4, 5], [0, 1]):
            if all(
                ((val(sa, a, b) & val(sb, a, b)) ^ m) == ((tt >> (2 * a + b)) & 1)
                for a in (0, 1)
                for b in (0, 1)
            ):
                forms[tt] = (sa, sb, m)
                found = True
                break
        assert found, f"truth table {tt} not representable"
    return forms


_FORMS = _classify_gates()

# ---------------------------------------------------------------------------
# Device program.  call_plan: list of (slot_start, n_slots, num_idxs_reg,
# is_tail); nslot_n / nslot_x fixed by the plan.
# ---------------------------------------------------------------------------


def build_nc(call_plan, nslot_total, nslot_n):
    u32 = mybir.dt.uint32

    nc = bacc.Bacc(
        "TRN2", target_bir_lowering=False, debug=False, num_devices=NCORES
    )
    tab = nc.dram_tensor("tab", [ROWS, BPACK], mybir.dt.uint8, kind="ExternalInput")
    total_idx_cols = sum(2 * n * 128 // 16 for _, n, _, _ in call_plan)
    idxs = nc.dram_tensor(
        "idxs", [128, total_idx_cols], mybir.dt.int16, kind="ExternalInput"
    )
    cst = nc.dram_tensor("cst", [128, nslot_n, 4], mybir.dt.uint8, kind="ExternalInput")
    outd = nc.dram_tensor(
        "out", [128, nslot_total * BPACK], mybir.dt.uint8, kind="ExternalOutput"
    )

    with tile.TileContext(nc) as tc, ExitStack() as ctx:
        pconst = ctx.enter_context(tc.tile_pool(name="const", bufs=1))
        pg = ctx.enter_context(tc.tile_pool(name="gather", bufs=1))
        pq = ctx.enter_context(tc.tile_pool(name="and", bufs=1))
        po = ctx.enter_context(tc.tile_pool(name="out", bufs=1))

        idx_t = pconst.tile([128, total_idx_cols], mybir.dt.int16)
        nc.sync.dma_start(idx_t[:], idxs[:])
        cst_t = pconst.tile([128, nslot_n, 4], mybir.dt.uint8)
        nc.sync.dma_start(cst_t[:], cst[:])

        icol = 0
        for k, (s0, n, nreg, is_tail) in enumerate(call_plan):
            ncols = 2 * n * 128 // 16
            g_t = pg.tile([128, 2 * n, BPACK], mybir.dt.uint8, tag=f"g{k}")
            nc.gpsimd.dma_gather(
                g_t[:],
                tab[:],
                idx_t[:, icol : icol + ncols],
                2 * n * 128,
                nreg,
                BPACK,
                single_packet=False,
            )
            icol += ncols
            o_t = po.tile([128, n, BPACK], mybir.dt.uint8, tag=f"o{k}")
            if is_tail:
                nc.vector.tensor_tensor(
                    o_t[:].bitcast(u32),
                    g_t[:, 0:n, :].bitcast(u32),
                    g_t[:, n : 2 * n, :].bitcast(u32),
                    mybir.AluOpType.bitwise_xor,
                )
            else:
                q_t = pq.tile([128, n, BPACK], mybir.dt.uint8, tag=f"q{k}")
                nc.vector.tensor_tensor(
                    q_t[:].bitcast(u32),
                    g_t[:, 0:n, :].bitcast(u32),
                    g_t[:, n : 2 * n, :].bitcast(u32),
                    mybir.AluOpType.bitwise_and,
                )
                nc.vector.tensor_tensor(
                    o_t[:].bitcast(u32),
                    q_t[:].bitcast(u32),
                    cst_t[:, s0 : s0 + n, :]
                    .bitcast(u32)
                    .broadcast_to([128, n, BPACK // 4]),
                    mybir.AluOpType.bitwise_xor,
                )
            nc.sync.dma_start(
                outd[:, s0 * BPACK : (s0 + n) * BPACK], o_t[:]
            )
    nc.compile()
    return nc


# ---------------------------------------------------------------------------
# Host-side input prep / output assembly
# ---------------------------------------------------------------------------


def _prep(x, gates, choices):
    x8 = np.asarray(x, dtype=np.uint8)
    g8 = np.asarray(gates, dtype=np.uint8)
    ch = np.asarray(choices, dtype=np.int64)

    xp = np.packbits(x8, axis=0, bitorder="little")  # [B/8, N]
    tabx = np.ascontiguousarray(xp.T)                # [N, BPACK]
    tab_full = np.empty((ROWS, BPACK), dtype=np.uint8)
    tab_full[:N] = tabx
    tab_full[N : 2 * N] = tabx ^ 0xFF
    tab_full[2 * N] = 0xFF
    tab_full[ZROW] = 0

    tt = (g8[:, 0] | (g8[:, 1] << 1) | (g8[:, 2] << 2) | (g8[:, 3] << 3)).astype(
        np.int64
    )
    sel = _FORMS[tt]                       # [G, 3] (sa, sb, m|2)
    c0, c1 = ch[:, 0], ch[:, 1]

    def row_of(code):
        return np.select(
            [code == 0, code == 1, code == 2, code == 3, code == 4, code == 5],
            [c0, N + c0, c1, N + c1,
             np.full(G, 2 * N, np.int64), np.full(G, ZROW, np.int64)],
        )

    rowA = row_of(sel[:, 0])
    rowB = row_of(sel[:, 1])
    is_x = sel[:, 2] == 2
    mask = (sel[:, 2] == 1).astype(np.uint8) * 0xFF

    # per-core reordered gate lists
    norm_ids, xor_ids = [], []
    for c in range(NCORES):
        gl = np.arange(c * GPC, (c + 1) * GPC)
        norm_ids.append(gl[~is_x[gl]])
        xor_ids.append(gl[is_x[gl]])
    nn = np.array([len(v) for v in norm_ids])
    nx = np.array([len(v) for v in xor_ids])

    nslot_n = int(np.ceil(nn.max() / 128))
    nslot_x = int(np.ceil(nx.max() / 128))
    nslot_total = nslot_n + nslot_x

    # call plan (shared across cores): normal calls + one tail call
    call_plan = []
    for s0, n in NORM_CALLS:
        assert s0 + n <= nslot_n
        cap = n * 128
        # real (non-pad) A entries in this call, equalized across cores
        rmax = int(min(cap, max(0, (nn - s0 * 128).max())))
        full_a = cap  # A-block fully valid (pads use the zeros row)
        nreg = full_a + rmax
        call_plan.append((s0, n, nreg, False))
    capx = nslot_x * 128
    rxmax = int(nx.max())
    call_plan.append((nslot_n, nslot_x, capx + rxmax, True))

    in_maps = []
    for c in range(NCORES):
        na, nxr = nn[c], nx[c]
        # per-position rows for the reordered layout
        posA = np.full(nslot_n * 128, ZROW, np.int64)
        posB = np.full(nslot_n * 128, ZROW, np.int64)
        posA[:na] = rowA[norm_ids[c]]
        posB[:na] = rowB[norm_ids[c]]
        xposA = np.full(capx, ZROW, np.int64)
        xposB = np.full(capx, ZROW, np.int64)
        xposA[:nxr] = rowA[xor_ids[c]]
        xposB[:nxr] = rowB[xor_ids[c]]

        cols = []
        for s0, n, nreg, is_tail in call_plan:
            cap = n * 128
            if is_tail:
                a, b = xposA, xposB
                lo = 0
            else:
                a, b = posA, posB
                lo = s0 * 128
            flat = np.full(2 * cap, -1, np.int64)
            flat[:cap] = a[lo : lo + cap]
            nreal = nreg - cap  # equalized count of valid B entries
            flat[cap : cap + nreal] = b[lo : lo + nreal]
            # positions beyond per-core real gates already hold ZROW pads
            wrapped = flat.astype(np.int16).reshape(-1, 16).T
            cols.append(np.tile(wrapped, (8, 1)))
        idxs_np = np.ascontiguousarray(np.concatenate(cols, axis=1))

        mcol = np.zeros(nslot_n * 128, np.uint8)
        mcol[:na] = mask[norm_ids[c]]
        cst_np = np.repeat(
            mcol.reshape(nslot_n, 128).T[:, :, None], 4, axis=2
        )  # [128, nslot_n, 4]
        in_maps.append(
            {
                "tab": tab_full,
                "idxs": idxs_np,
                "cst": np.ascontiguousarray(cst_np),
            }
        )

    meta = {
        "call_plan": tuple(call_plan),
        "nslot_n": nslot_n,
        "nslot_total": nslot_total,
        "norm_ids": norm_ids,
        "xor_ids": xor_ids,
    }
    return in_maps, meta


def _assemble(results, meta):
    nslot_n = meta["nslot_n"]
    nslot_total = meta["nslot_total"]
    packed = np.empty((G, BPACK), np.uint8)
    for c in range(NCORES):
        o = results[c]["out"]  # [128, nslot_total*BPACK]
        pos = (
            o.reshape(128, nslot_total, BPACK).transpose(1, 0, 2).reshape(-1, BPACK)
        )
        nids, xids = meta["norm_ids"][c], meta["xor_ids"][c]
        packed[nids] = pos[: len(nids)]
        packed[xids] = pos[nslot_n * 128 : nslot_n * 128 + len(xids)]
    return np.unpackbits(packed, axis=1, bitorder="little").T.astype(bool)


# ---------------------------------------------------------------------------
# Entry point
# ---------------------------------------------------------------------------

_NC_CACHE = {}


def _get_nc(call_plan, nslot_total, nslot_n):
    key = (tuple(call_plan), nslot_total, nslot_n)
    if key not in _NC_CACHE:
        _NC_CACHE[key] = build_nc(call_plan, nslot_total, nslot_n)
    return _NC_CACHE[key]


def kernel(x, gates, choices):
    in_maps, meta = _prep(x, gates, choices)
    nc = _get_nc(meta["call_plan"], meta["nslot_total"], meta["nslot_n"])
    res = run_bass_kernel_spmd(nc, in_maps, list(range(NCORES)))
    return _assemble(res.results, meta)


# revision 10
# speedup vs baseline: 1.3524x; 1.1357x over previous
"""GateRow kernel for Trainium2 (8 NeuronCores, SPMD gate-sharded).

Problem: out[b, g] = gates[g, 2*x[b, c0[g]] + x[b, c1[g]]]
  x: [16384, 8192] bool, gates: [8192, 4] bool, choices: [8192, 2] int32.

Strategy (per core, gate-sharded GPC=1024, batch bit-packed 8 bits/byte):
  host:  TAB = [packbits(x)^T ; ~packbits(x)^T ; ones ; zeros]
         (16386 rows x 2048 bytes).  Classify each gate:
           AND/OR class (14/16 truth tables): f = (a' & b') ^ m,
             a'/b' TAB rows, m per-gate constant byte mask (0x00/0xFF)
           XOR class (tt 0110/1001): f = a' ^ b'
         XOR-class gates are moved to dedicated tail slots per core.
  device:
    normal slots:  dma_gather a'|b' rows -> q = a'&b' -> o = q ^ mask
                   (mask via stride-0 broadcast of a per-partition u32)
    tail slots:    dma_gather a'|b' rows -> o = a'^b'
    all bitwise ops as uint32 lanes on DVE; packed rows DMA'd out.
  host:  unpack bits + transpose to [B, G] bool.

Descriptor count ~2.3k/core (was 3.1k in the 3-row universal form); DMA
~6.7 MB/core.  Pool-engine SWDGE descriptor generation is the serial
backbone, so index arrays are equalized across cores (shared
num_idxs_reg) with trailing -1 entries that generate no descriptors.
"""

import sys

for _p in ("/opt/trn_rl_repo", "/opt/pypackages"):
    if _p not in sys.path:
        sys.path.append(_p)

from contextlib import ExitStack
from itertools import product

import numpy as np

import concourse.bass as bass  # noqa: F401
import concourse.bacc as bacc
import concourse.tile as tile
import concourse.mybir as mybir
from concourse.bass_utils import run_bass_kernel_spmd

B, N, G, NCORES = 16384, 8192, 8192, 8
GPC = G // NCORES      # 1024 gates per core
BPACK = B // 8         # 2048 packed bytes per table row
ROWS = 2 * N + 2       # x rows, ~x rows, ones, zeros
ZROW = 2 * N + 1       # all-zeros row (pad target)

# XOR-expressible truth tables: f = a' ^ b' with rows from TAB
#   {tt: (sa, sb)}; codes 0=a,1=~a,2=b,3=~b,4=ones,5=zeros


def _xor_forms():
    def val(sel, a, b):
        return [a, 1 - a, b, 1 - b, 1, 0][sel]

    forms = {}
    for tt in range(16):
        for sa, sb in product(range(6), range(6)):
            if all(
                (val(sa, a, b) ^ val(sb, a, b)) == ((tt >> (2 * a + b)) & 1)
                for a in (0, 1)
                for b in (0, 1)
            ):
                forms[tt] = (sa, sb)
                break
    return forms


_XFORMS = _xor_forms()

# ---------------------------------------------------------------------------
# Gate classification
#   AND/OR class: f(a,b) = (a' & b') ^ m,  a' in {a,~a,1,0}, b' in {b,~b,1,0}
#   XOR class (tt 6/9): f = a' ^ b'
#   selector codes: 0 = a, 1 = ~a, 2 = b, 3 = ~b, 4 = ones, 5 = zeros
# ---------------------------------------------------------------------------


def _classify_gates():
    def val(sel, a, b):
        return [a, 1 - a, b, 1 - b, 1, 0][sel]

    forms = np.full((16, 3), -1, dtype=np.int64)  # (sa, sb, m) ; m==2 -> XOR class
    for tt in range(16):
        if tt in (6, 9):
            # a ^ b   /   ~a ^ b
            forms[tt] = (0 if tt == 6 else 1, 2, 2)
            continue
        found = False
        for sa, sb, m in product([0, 1, 4, 5], [2, 3, 4, 5], [0, 1]):
            if all(
                ((val(sa, a, b) & val(sb, a, b)) ^ m) == ((tt >> (2 * a + b)) & 1)
                for a in (0, 1)
                for b in (0, 1)
            ):
                forms[tt] = (sa, sb, m)
                found = True
                break
        assert found, f"truth table {tt} not representable"
    return forms


_FORMS = _classify_gates()

# ---------------------------------------------------------------------------
# Device program.  call_plan: list of (slot_start, n_slots, num_idxs_reg,
# is_tail); nslot_n / nslot_x fixed by the plan.
# ---------------------------------------------------------------------------


def build_nc(call_plan, nslot_total, nslot_n):
    u32 = mybir.dt.uint32

    nc = bacc.Bacc(
        "TRN2", target_bir_lowering=False, debug=False, num_devices=NCORES
    )
    tab = nc.dram_tensor("tab", [ROWS, BPACK], mybir.dt.uint8, kind="ExternalInput")
    total_idx_cols = sum(2 * n * 128 // 16 for _, n, _, _ in call_plan)
    idxs = nc.dram_tensor(
        "idxs", [128, total_idx_cols], mybir.dt.int16, kind="ExternalInput"
    )
    cst = nc.dram_tensor("cst", [128, nslot_n, 4], mybir.dt.uint8, kind="ExternalInput")
    outd = nc.dram_tensor(
        "out", [128, nslot_total * BPACK], mybir.dt.uint8, kind="ExternalOutput"
    )

    with tile.TileContext(nc) as tc, ExitStack() as ctx:
        pconst = ctx.enter_context(tc.tile_pool(name="const", bufs=1))
        pg = ctx.enter_context(tc.tile_pool(name="gather", bufs=1))
        pq = ctx.enter_context(tc.tile_pool(name="and", bufs=1))
        po = ctx.enter_context(tc.tile_pool(name="out", bufs=1))

        idx_t = pconst.tile([128, total_idx_cols], mybir.dt.int16)
        nc.sync.dma_start(idx_t[:], idxs[:])
        cst_t = pconst.tile([128, nslot_n, 4], mybir.dt.uint8)
        nc.sync.dma_start(cst_t[:], cst[:])

        icol = 0
        for k, (s0, n, nreg, is_tail) in enumerate(call_plan):
            ncols = 2 * n * 128 // 16
            g_t = pg.tile([128, 2 * n, BPACK], mybir.dt.uint8, tag=f"g{k}")
            nc.gpsimd.dma_gather(
                g_t[:],
                tab[:],
                idx_t[:, icol : icol + ncols],
                2 * n * 128,
                nreg,
                BPACK,
                single_packet=False,
            )
            icol += ncols
            o_t = po.tile([128, n, BPACK], mybir.dt.uint8, tag=f"o{k}")
            if is_tail:
                nc.vector.tensor_tensor(
                    o_t[:].bitcast(u32),
                    g_t[:, 0:n, :].bitcast(u32),
                    g_t[:, n : 2 * n, :].bitcast(u32),
                    mybir.AluOpType.bitwise_xor,
                )
            else:
                q_t = pq.tile([128, n, BPACK], mybir.dt.uint8, tag=f"q{k}")
                nc.vector.tensor_tensor(
                    q_t[:].bitcast(u32),
                    g_t[:, 0:n, :].bitcast(u32),
                    g_t[:, n : 2 * n, :].bitcast(u32),
                    mybir.AluOpType.bitwise_and,
                )
                nc.vector.tensor_tensor(
                    o_t[:].bitcast(u32),
                    q_t[:].bitcast(u32),
                    cst_t[:, s0 : s0 + n, :]
                    .bitcast(u32)
                    .broadcast_to([128, n, BPACK // 4]),
                    mybir.AluOpType.bitwise_xor,
                )
            nc.sync.dma_start(
                outd[:, s0 * BPACK : (s0 + n) * BPACK], o_t[:]
            )
    nc.compile()
    return nc


# ---------------------------------------------------------------------------
# Host-side input prep / output assembly
# ---------------------------------------------------------------------------


def _prep(x, gates, choices):
    x8 = np.asarray(x, dtype=np.uint8)
    g8 = np.asarray(gates, dtype=np.uint8)
    ch = np.asarray(choices, dtype=np.int64)

    xp = np.packbits(x8, axis=0, bitorder="little")  # [B/8, N]
    tabx = np.ascontiguousarray(xp.T)                # [N, BPACK]
    tab_full = np.empty((ROWS, BPACK), dtype=np.uint8)
    tab_full[:N] = tabx
    tab_full[N : 2 * N] = tabx ^ 0xFF
    tab_full[2 * N] = 0xFF
    tab_full[ZROW] = 0

    tt = (g8[:, 0] | (g8[:, 1] << 1) | (g8[:, 2] << 2) | (g8[:, 3] << 3)).astype(
        np.int64
    )
    sel = _FORMS[tt]                       # [G, 3] (sa, sb, m|2)
    c0, c1 = ch[:, 0], ch[:, 1]

    def row_of(code):
        return np.select(
            [code == 0, code == 1, code == 2, code == 3, code == 4, code == 5],
            [c0, N + c0, c1, N + c1,
             np.full(G, 2 * N, np.int64), np.full(G, ZROW, np.int64)],
        )

    rowA = row_of(sel[:, 0])
    rowB = row_of(sel[:, 1])
    is_x = sel[:, 2] == 2
    mask = (sel[:, 2] == 1).astype(np.uint8) * 0xFF

    # XOR-form rows (valid for tts in _XFORMS): used for gates placed in the
    # tail region, including degenerate gates moved there for rebalancing.
    xsel = np.zeros((G, 2), np.int64)
    can_x = np.zeros(G, bool)
    for t, (sa, sb) in _XFORMS.items():
        m = tt == t
        xsel[m] = (sa, sb)
        can_x[m] = True
    xrowA = row_of(xsel[:, 0])
    xrowB = row_of(xsel[:, 1])

    # per-core gate lists: rebalance so the normal region fits 7 slots by
    # moving degenerate (XOR-expressible) gates into the tail region
    norm_ids, xor_ids = [], []
    for c in range(NCORES):
        gl = np.arange(c * GPC, (c + 1) * GPC)
        nrm = gl[~is_x[gl]]
        xr = list(gl[is_x[gl]])
        spill = len(nrm) - 7 * 128
        if spill > 0:
            movable = nrm[can_x[nrm]]
            assert len(movable) >= spill, "not enough degenerate gates to move"
            mv = set(movable[:spill].tolist())
            xr += sorted(mv)
            nrm = np.array([g for g in nrm if g not in mv])
        norm_ids.append(nrm)
        xor_ids.append(np.array(sorted(xr), dtype=np.int64))
    nn = np.array([len(v) for v in norm_ids])
    nx = np.array([len(v) for v in xor_ids])

    nslot_n = int(np.ceil(nn.max() / 128))
    nslot_x = int(np.ceil(nx.max() / 128))
    nslot_total = nslot_n + nslot_x

    # call plan (shared across cores); all index positions valid (zeros-row
    # pads) so num_idxs_reg == num_idxs == capacity on every core
    norm_calls = ((0, 2), (2, 2), (4, nslot_n - 4))
    call_plan = []
    for s0, n in norm_calls:
        assert s0 + n <= nslot_n
        call_plan.append((s0, n, 2 * n * 128, False))
    call_plan.append((nslot_n, nslot_x, 2 * nslot_x * 128, True))

    in_maps = []
    for c in range(NCORES):
        na, nxr = nn[c], nx[c]
        # per-position rows for the reordered layout
        posA = np.full(nslot_n * 128, ZROW, np.int64)
        posB = np.full(nslot_n * 128, ZROW, np.int64)
        posA[:na] = rowA[norm_ids[c]]
        posB[:na] = rowB[norm_ids[c]]
        capx = nslot_x * 128
        xposA = np.full(capx, ZROW, np.int64)
        xposB = np.full(capx, ZROW, np.int64)
        xposA[:nxr] = xrowA[xor_ids[c]]
        xposB[:nxr] = xrowB[xor_ids[c]]

        cols = []
        for s0, n, nreg, is_tail in call_plan:
            cap = n * 128
            if is_tail:
                a, b = xposA, xposB
                lo = 0
            else:
                a, b = posA, posB
                lo = s0 * 128
            flat = np.concatenate([a[lo : lo + cap], b[lo : lo + cap]])
            wrapped = flat.astype(np.int16).reshape(-1, 16).T
            cols.append(np.tile(wrapped, (8, 1)))
        idxs_np = np.ascontiguousarray(np.concatenate(cols, axis=1))

        mcol = np.zeros(nslot_n * 128, np.uint8)
        mcol[:na] = mask[norm_ids[c]]
        cst_np = np.repeat(
            mcol.reshape(nslot_n, 128).T[:, :, None], 4, axis=2
        )  # [128, nslot_n, 4]
        in_maps.append(
            {
                "tab": tab_full,
                "idxs": idxs_np,
                "cst": np.ascontiguousarray(cst_np),
            }
        )

    meta = {
        "call_plan": tuple(call_plan),
        "nslot_n": nslot_n,
        "nslot_total": nslot_total,
        "norm_ids": norm_ids,
        "xor_ids": xor_ids,
    }
    return in_maps, meta


def _assemble(results, meta):
    nslot_n = meta["nslot_n"]
    nslot_total = meta["nslot_total"]
    packed = np.empty((G, BPACK), np.uint8)
    for c in range(NCORES):
        o = results[c]["out"]  # [128, nslot_total*BPACK]
        pos = (
            o.reshape(128, nslot_total, BPACK).transpose(1, 0, 2).reshape(-1, BPACK)
        )
        nids, xids = meta["norm_ids"][c], meta["xor_ids"][c]
        packed[nids] = pos[: len(nids)]
        packed[xids] = pos[nslot_n * 128 : nslot_n * 128 + len(xids)]
    return np.unpackbits(packed, axis=1, bitorder="little").T.astype(bool)


# ---------------------------------------------------------------------------
# Entry point
# ---------------------------------------------------------------------------

_NC_CACHE = {}


def _get_nc(call_plan, nslot_total, nslot_n):
    key = (tuple(call_plan), nslot_total, nslot_n)
    if key not in _NC_CACHE:
        _NC_CACHE[key] = build_nc(call_plan, nslot_total, nslot_n)
    return _NC_CACHE[key]


def kernel(x, gates, choices):
    in_maps, meta = _prep(x, gates, choices)
    nc = _get_nc(meta["call_plan"], meta["nslot_total"], meta["nslot_n"])
    res = run_bass_kernel_spmd(nc, in_maps, list(range(NCORES)))
    return _assemble(res.results, meta)


# revision 11
# speedup vs baseline: 1.4266x; 1.0549x over previous
"""GateRow kernel for Trainium2 (8 NeuronCores, SPMD gate-sharded).

Problem: out[b, g] = gates[g, 2*x[b, c0[g]] + x[b, c1[g]]]
  x: [16384, 8192] bool, gates: [8192, 4] bool, choices: [8192, 2] int32.

Strategy (per core, gate-sharded GPC=1024, batch bit-packed 8 bits/byte):
  host:  TAB = [packbits(x)^T ; ~packbits(x)^T ; ones ; zeros]
         (16386 rows x 2048 bytes).  Classify each gate:
           AND/OR class (14/16 truth tables): f = (a' & b') ^ m,
             a'/b' TAB rows, m per-gate constant byte mask (0x00/0xFF)
           XOR class (tt 0110/1001): f = a' ^ b'
         XOR-class gates are moved to dedicated tail slots per core.
  device:
    normal slots:  dma_gather a'|b' rows -> q = a'&b' -> o = q ^ mask
                   (mask via stride-0 broadcast of a per-partition u32)
    tail slots:    dma_gather a'|b' rows -> o = a'^b'
    all bitwise ops as uint32 lanes on DVE; packed rows DMA'd out.
  host:  unpack bits + transpose to [B, G] bool.

Descriptor count ~2.3k/core (was 3.1k in the 3-row universal form); DMA
~6.7 MB/core.  Pool-engine SWDGE descriptor generation is the serial
backbone, so index arrays are equalized across cores (shared
num_idxs_reg) with trailing -1 entries that generate no descriptors.
"""

import sys

for _p in ("/opt/trn_rl_repo", "/opt/pypackages"):
    if _p not in sys.path:
        sys.path.append(_p)

from contextlib import ExitStack
from itertools import product

import numpy as np

import concourse.bass as bass  # noqa: F401
import concourse.bacc as bacc
import concourse.tile as tile
import concourse.mybir as mybir
from concourse.bass_utils import run_bass_kernel_spmd

B, N, G, NCORES = 16384, 8192, 8192, 8
GPC = G // NCORES      # 1024 gates per core
BPACK = B // 8         # 2048 packed bytes per table row
ROWS = 2 * N + 2       # x rows, ~x rows, ones, zeros
ZROW = 2 * N + 1       # all-zeros row (pad target)

# XOR-expressible truth tables: f = a' ^ b' with rows from TAB
#   {tt: (sa, sb)}; codes 0=a,1=~a,2=b,3=~b,4=ones,5=zeros


def _xor_forms():
    def val(sel, a, b):
        return [a, 1 - a, b, 1 - b, 1, 0][sel]

    forms = {}
    for tt in range(16):
        for sa, sb in product(range(6), range(6)):
            if all(
                (val(sa, a, b) ^ val(sb, a, b)) == ((tt >> (2 * a + b)) & 1)
                for a in (0, 1)
                for b in (0, 1)
            ):
                forms[tt] = (sa, sb)
                break
    return forms


_XFORMS = _xor_forms()

# ---------------------------------------------------------------------------
# Gate classification
#   AND/OR class: f(a,b) = (a' & b') ^ m,  a' in {a,~a,1,0}, b' in {b,~b,1,0}
#   XOR class (tt 6/9): f = a' ^ b'
#   selector codes: 0 = a, 1 = ~a, 2 = b, 3 = ~b, 4 = ones, 5 = zeros
# ---------------------------------------------------------------------------


def _classify_gates():
    def val(sel, a, b):
        return [a, 1 - a, b, 1 - b, 1, 0][sel]

    forms = np.full((16, 3), -1, dtype=np.int64)  # (sa, sb, m) ; m==2 -> XOR class
    for tt in range(16):
        if tt in (6, 9):
            # a ^ b   /   ~a ^ b
            forms[tt] = (0 if tt == 6 else 1, 2, 2)
            continue
        found = False
        for sa, sb, m in product([0, 1, 4, 5], [2, 3, 4, 5], [0, 1]):
            if all(
                ((val(sa, a, b) & val(sb, a, b)) ^ m) == ((tt >> (2 * a + b)) & 1)
                for a in (0, 1)
                for b in (0, 1)
            ):
                forms[tt] = (sa, sb, m)
                found = True
                break
        assert found, f"truth table {tt} not representable"
    return forms


_FORMS = _classify_gates()

# ---------------------------------------------------------------------------
# Device program.  call_plan: list of (slot_start, n_slots, num_idxs_reg,
# is_tail); nslot_n / nslot_x fixed by the plan.
# ---------------------------------------------------------------------------


def build_nc(call_plan, nslot_total, nslot_n):
    u32 = mybir.dt.uint32

    nc = bacc.Bacc(
        "TRN2", target_bir_lowering=False, debug=False, num_devices=NCORES
    )
    tab = nc.dram_tensor("tab", [ROWS, BPACK], mybir.dt.uint8, kind="ExternalInput")
    total_idx_cols = sum(2 * n * 128 // 16 for _, n, _, _ in call_plan)
    idxs = nc.dram_tensor(
        "idxs", [128, total_idx_cols], mybir.dt.int16, kind="ExternalInput"
    )
    cst = nc.dram_tensor("cst", [128, nslot_n, 4], mybir.dt.uint8, kind="ExternalInput")
    outd = nc.dram_tensor(
        "out", [128, nslot_total * BPACK], mybir.dt.uint8, kind="ExternalOutput"
    )

    with tile.TileContext(nc) as tc, ExitStack() as ctx:
        pconst = ctx.enter_context(tc.tile_pool(name="const", bufs=1))
        pg = ctx.enter_context(tc.tile_pool(name="gather", bufs=1))
        pq = ctx.enter_context(tc.tile_pool(name="and", bufs=1))
        po = ctx.enter_context(tc.tile_pool(name="out", bufs=1))

        idx_t = pconst.tile([128, total_idx_cols], mybir.dt.int16)
        nc.sync.dma_start(idx_t[:], idxs[:])
        cst_t = pconst.tile([128, nslot_n, 4], mybir.dt.uint8)
        nc.sync.dma_start(cst_t[:], cst[:])

        icol = 0
        for k, (s0, n, nreg, is_tail) in enumerate(call_plan):
            ncols = 2 * n * 128 // 16
            g_t = pg.tile([128, 2 * n, BPACK], mybir.dt.uint8, tag=f"g{k}")
            nc.gpsimd.dma_gather(
                g_t[:],
                tab[:],
                idx_t[:, icol : icol + ncols],
                2 * n * 128,
                nreg,
                BPACK,
                single_packet=False,
            )
            icol += ncols
            o_t = po.tile([128, n, BPACK], mybir.dt.uint8, tag=f"o{k}")
            if is_tail:
                nc.vector.tensor_tensor(
                    o_t[:].bitcast(u32),
                    g_t[:, 0:n, :].bitcast(u32),
                    g_t[:, n : 2 * n, :].bitcast(u32),
                    mybir.AluOpType.bitwise_xor,
                )
            else:
                q_t = pq.tile([128, n, BPACK], mybir.dt.uint8, tag=f"q{k}")
                nc.vector.tensor_tensor(
                    q_t[:].bitcast(u32),
                    g_t[:, 0:n, :].bitcast(u32),
                    g_t[:, n : 2 * n, :].bitcast(u32),
                    mybir.AluOpType.bitwise_and,
                )
                nc.vector.tensor_tensor(
                    o_t[:].bitcast(u32),
                    q_t[:].bitcast(u32),
                    cst_t[:, s0 : s0 + n, :]
                    .bitcast(u32)
                    .broadcast_to([128, n, BPACK // 4]),
                    mybir.AluOpType.bitwise_xor,
                )
            nc.sync.dma_start(
                outd[:, s0 * BPACK : (s0 + n) * BPACK], o_t[:]
            )
    nc.compile()
    return nc


# ---------------------------------------------------------------------------
# Host-side input prep / output assembly
# ---------------------------------------------------------------------------


def _prep(x, gates, choices):
    x8 = np.asarray(x, dtype=np.uint8)
    g8 = np.asarray(gates, dtype=np.uint8)
    ch = np.asarray(choices, dtype=np.int64)

    xp = np.packbits(x8, axis=0, bitorder="little")  # [B/8, N]
    tabx = np.ascontiguousarray(xp.T)                # [N, BPACK]
    tab_full = np.empty((ROWS, BPACK), dtype=np.uint8)
    tab_full[:N] = tabx
    tab_full[N : 2 * N] = tabx ^ 0xFF
    tab_full[2 * N] = 0xFF
    tab_full[ZROW] = 0

    tt = (g8[:, 0] | (g8[:, 1] << 1) | (g8[:, 2] << 2) | (g8[:, 3] << 3)).astype(
        np.int64
    )
    sel = _FORMS[tt]                       # [G, 3] (sa, sb, m|2)
    c0, c1 = ch[:, 0], ch[:, 1]

    def row_of(code):
        return np.select(
            [code == 0, code == 1, code == 2, code == 3, code == 4, code == 5],
            [c0, N + c0, c1, N + c1,
             np.full(G, 2 * N, np.int64), np.full(G, ZROW, np.int64)],
        )

    rowA = row_of(sel[:, 0])
    rowB = row_of(sel[:, 1])
    is_x = sel[:, 2] == 2
    mask = (sel[:, 2] == 1).astype(np.uint8) * 0xFF

    # XOR-form rows (valid for tts in _XFORMS): used for gates placed in the
    # tail region, including degenerate gates moved there for rebalancing.
    xsel = np.zeros((G, 2), np.int64)
    can_x = np.zeros(G, bool)
    for t, (sa, sb) in _XFORMS.items():
        m = tt == t
        xsel[m] = (sa, sb)
        can_x[m] = True
    xrowA = row_of(xsel[:, 0])
    xrowB = row_of(xsel[:, 1])

    # per-core gate lists: move exactly (256 - n_xor) degenerate
    # (XOR-expressible) gates into the tail region so every core has
    # exactly 768 normal + 256 tail gates -- 8 slots, zero padding.
    nslot_n, nslot_x = 6, 2
    norm_ids, xor_ids = [], []
    for c in range(NCORES):
        gl = np.arange(c * GPC, (c + 1) * GPC)
        nrm = gl[~is_x[gl]]
        xr = list(gl[is_x[gl]])
        spill = GPC - nslot_n * 128 - len(xr)
        assert spill >= 0, "more XOR-class gates than tail capacity"
        if spill > 0:
            movable = nrm[can_x[nrm]]
            assert len(movable) >= spill, "not enough degenerate gates to move"
            mv = set(movable[:spill].tolist())
            xr += sorted(mv)
            nrm = np.array([g for g in nrm if g not in mv])
        norm_ids.append(nrm)
        xor_ids.append(np.array(sorted(xr), dtype=np.int64))
        assert len(nrm) == nslot_n * 128 and len(xor_ids[-1]) == nslot_x * 128
    nn = np.array([len(v) for v in norm_ids])
    nx = np.array([len(v) for v in xor_ids])
    nslot_total = nslot_n + nslot_x

    # call plan (shared across cores): tail first, small normal call last
    call_plan = [(nslot_n, nslot_x, 2 * nslot_x * 128, True)]
    for s0, n in ((0, 3), (3, 2), (5, 1)):
        call_plan.append((s0, n, 2 * n * 128, False))

    in_maps = []
    for c in range(NCORES):
        na, nxr = nn[c], nx[c]
        # per-position rows for the reordered layout
        posA = np.full(nslot_n * 128, ZROW, np.int64)
        posB = np.full(nslot_n * 128, ZROW, np.int64)
        posA[:na] = rowA[norm_ids[c]]
        posB[:na] = rowB[norm_ids[c]]
        capx = nslot_x * 128
        xposA = np.full(capx, ZROW, np.int64)
        xposB = np.full(capx, ZROW, np.int64)
        xposA[:nxr] = xrowA[xor_ids[c]]
        xposB[:nxr] = xrowB[xor_ids[c]]

        cols = []
        for s0, n, nreg, is_tail in call_plan:
            cap = n * 128
            if is_tail:
                a, b = xposA, xposB
                lo = 0
            else:
                a, b = posA, posB
                lo = s0 * 128
            flat = np.concatenate([a[lo : lo + cap], b[lo : lo + cap]])
            wrapped = flat.astype(np.int16).reshape(-1, 16).T
            cols.append(np.tile(wrapped, (8, 1)))
        idxs_np = np.ascontiguousarray(np.concatenate(cols, axis=1))

        mcol = np.zeros(nslot_n * 128, np.uint8)
        mcol[:na] = mask[norm_ids[c]]
        cst_np = np.repeat(
            mcol.reshape(nslot_n, 128).T[:, :, None], 4, axis=2
        )  # [128, nslot_n, 4]
        in_maps.append(
            {
                "tab": tab_full,
                "idxs": idxs_np,
                "cst": np.ascontiguousarray(cst_np),
            }
        )

    meta = {
        "call_plan": tuple(call_plan),
        "nslot_n": nslot_n,
        "nslot_total": nslot_total,
        "norm_ids": norm_ids,
        "xor_ids": xor_ids,
    }
    return in_maps, meta


def _assemble(results, meta):
    nslot_n = meta["nslot_n"]
    nslot_total = meta["nslot_total"]
    packed = np.empty((G, BPACK), np.uint8)
    for c in range(NCORES):
        o = results[c]["out"]  # [128, nslot_total*BPACK]
        pos = (
            o.reshape(128, nslot_total, BPACK).transpose(1, 0, 2).reshape(-1, BPACK)
        )
        nids, xids = meta["norm_ids"][c], meta["xor_ids"][c]
        packed[nids] = pos[: len(nids)]
        packed[xids] = pos[nslot_n * 128 : nslot_n * 128 + len(xids)]
    return np.unpackbits(packed, axis=1, bitorder="little").T.astype(bool)


# ---------------------------------------------------------------------------
# Entry point
# ---------------------------------------------------------------------------

_NC_CACHE = {}


def _get_nc(call_plan, nslot_total, nslot_n):
    key = (tuple(call_plan), nslot_total, nslot_n)
    if key not in _NC_CACHE:
        _NC_CACHE[key] = build_nc(call_plan, nslot_total, nslot_n)
    return _NC_CACHE[key]


def kernel(x, gates, choices):
    in_maps, meta = _prep(x, gates, choices)
    nc = _get_nc(meta["call_plan"], meta["nslot_total"], meta["nslot_n"])
    res = run_bass_kernel_spmd(nc, in_maps, list(range(NCORES)))
    return _assemble(res.results, meta)
